# revision 9
# baseline (speedup 1.0000x reference)
"""Distributed GNN neighbor-mean aggregation on 8 TRN2 NeuronCores — v3.

Same architecture as kernel2 (4-pass int16 dma_gather + PE f32 segment-sum)
with per-(chunk,pass)-segment gather instructions whose descriptor count is
trimmed to the 16-rounded max-over-cores pair count (Q7 descriptor
generation at ~8ns/descriptor is the bottleneck engine).
"""

import numpy as np

import concourse.bacc as bacc
import concourse.bass as bass
import concourse.mybir as mybir
import concourse.tile as tile
from concourse import bass_utils

N_NODES = 100000
S = 16
D = 128
N_CORES = 8
NPC = N_NODES // N_CORES  # 12500
P = 128
NCHUNK = 98
NPAD = NCHUNK * P
N_PASS = 4
PASS_ROWS = 25000

_f32 = mybir.dt.float32
_i16 = mybir.dt.int16

X = mybir.AxisListType.X
is_eq = mybir.AluOpType.is_equal


def build_program(bkq: np.ndarray, n16: np.ndarray) -> bass.Bass:
    """bkq: [NCHUNK, N_PASS] blocks per segment; n16: descriptor counts."""
    nblocks = int(bkq.sum())
    nstream = nblocks * P

    nc = bacc.Bacc("TRN2", target_bir_lowering=False, debug=False)
    feat = nc.dram_tensor("features", [N_NODES, D], _f32, kind="ExternalInput").ap()
    idx_d = nc.dram_tensor("idx_sb", [P, nstream // 16], _i16, kind="ExternalInput").ap()
    rel_d = nc.dram_tensor("rel_sb", [P, nblocks], _f32, kind="ExternalInput").ap()
    msk_d = nc.dram_tensor("msk_sb", [P, NCHUNK * S], _f32, kind="ExternalInput").ap()
    iota_d = nc.dram_tensor("iota_sb", [P, P], _f32, kind="ExternalInput").ap()
    out_d = nc.dram_tensor("out_sb", [NPAD, D], _f32, kind="ExternalOutput").ap()

    # segment offsets (blocks) in (q, k) order
    seg_blk_off = {}
    off = 0
    for q in range(N_PASS):
        for k in range(NCHUNK):
            seg_blk_off[(q, k)] = off
            off += int(bkq[k, q])
    assert off == nblocks

    with tile.TileContext(nc) as tc:
        with (
            tc.tile_pool(name="g", bufs=8) as gpool,
            tc.tile_pool(name="acc", bufs=1) as apool,
            tc.tile_pool(name="pre", bufs=1) as ppool,
            tc.tile_pool(name="seg", bufs=8) as segpool,
            tc.tile_pool(name="ob", bufs=3) as opool,
            tc.tile_pool(name="ps", bufs=6, space="PSUM") as pspool,
        ):
            idx_t = ppool.tile([P, nstream // 16], _i16)
            rel_t = ppool.tile([P, nblocks], _f32)
            msk_t = ppool.tile([P, NCHUNK * S], _f32)
            iota_t = ppool.tile([P, P], _f32)
            nc.sync.dma_start(out=idx_t[:], in_=idx_d[:, :])
            nc.sync.dma_start(out=rel_t[:], in_=rel_d[:, :])
            nc.sync.dma_start(out=msk_t[:], in_=msk_d[:, :])
            nc.sync.dma_start(out=iota_t[:], in_=iota_d[:, :])

            cnt = ppool.tile([P, NCHUNK], _f32)
            invc = ppool.tile([P, NCHUNK], _f32)
            nc.vector.reduce_sum(
                out=cnt[:], in_=msk_t[:].rearrange("p (k j) -> p k j", j=S), axis=X
            )
            nc.vector.tensor_scalar_max(out=invc[:], in0=cnt[:], scalar1=1.0)
            nc.vector.reciprocal(out=invc[:], in_=invc[:])

            acc = apool.tile([P, NCHUNK * D], _f32)

            for q in range(N_PASS):
                for k in range(NCHUNK):
                    b = int(bkq[k, q])
                    s0 = seg_blk_off[(q, k)]
                    ndesc = int(n16[k, q])
                    gbuf = gpool.tile([P, b * D], _f32, tag="g")
                    nc.gpsimd.dma_gather(
                        out_ap=gbuf[:].rearrange("p (b d) -> p b d", d=D),
                        in_ap=feat[q * PASS_ROWS : (q + 1) * PASS_ROWS, :],
                        idxs_ap=idx_t[:, s0 * 8 : s0 * 8 + ndesc // 16],
                        num_idxs=ndesc,
                        num_idxs_reg=ndesc,
                        elem_size=D,
                        single_packet=False,
                    )
                    ps = pspool.tile([P, D], _f32, tag="ps", space="PSUM")
                    for pos in range(b):
                        seg_t = segpool.tile([P, P], _f32, tag="seg")
                        nc.vector.tensor_scalar(
                            out=seg_t[:],
                            in0=iota_t[:],
                            scalar1=rel_t[:, s0 + pos : s0 + pos + 1],
                            scalar2=None,
                            op0=is_eq,
                        )
                        nc.tensor.matmul(
                            out=ps[:],
                            lhsT=seg_t[:],
                            rhs=gbuf[:, pos * D : (pos + 1) * D],
                            start=pos == 0,
                            stop=pos == b - 1,
                        )
                    if q == 0:
                        nc.vector.tensor_copy(
                            out=acc[:, k * D : (k + 1) * D], in_=ps[:]
                        )
                    else:
                        nc.vector.tensor_add(
                            out=acc[:, k * D : (k + 1) * D],
                            in0=acc[:, k * D : (k + 1) * D],
                            in1=ps[:],
                        )
                    if q == N_PASS - 1:
                        ob = opool.tile([P, D], _f32, tag="ob")
                        nc.vector.tensor_scalar_mul(
                            out=ob[:],
                            in0=acc[:, k * D : (k + 1) * D],
                            scalar1=invc[:, k : k + 1],
                        )
                        nc.sync.dma_start(
                            out=out_d[k * P : (k + 1) * P, :], in_=ob[:]
                        )
    nc.finalize()
    return nc


def _marshal(features, neighbor_idx, neighbor_mask):
    feats = np.ascontiguousarray(features, dtype=np.float32)
    msk = np.asarray(neighbor_mask, dtype=bool)
    idx = np.asarray(neighbor_idx, dtype=np.int64)

    per_core = []
    counts_all = np.zeros((N_CORES, NCHUNK, N_PASS), np.int64)
    for c in range(N_CORES):
        sl = slice(c * NPC, (c + 1) * NPC)
        idx_c = idx[sl]
        msk_c = msk[sl]
        node_l, j = np.nonzero(msk_c)
        rows = idx_c[node_l, j]
        q = rows // PASS_ROWS
        k = node_l // P
        order = np.lexsort((node_l, k, q))
        node_s = node_l[order]
        rows_s = rows[order]
        q_s = q[order]
        k_s = k[order]
        cnt_kq = np.zeros((NCHUNK, N_PASS), np.int64)
        np.add.at(cnt_kq, (k_s, q_s), 1)
        counts_all[c] = cnt_kq
        per_core.append((node_s, rows_s, q_s, k_s, msk_c))

    maxcnt = counts_all.max(axis=0)  # [NCHUNK, N_PASS]
    n16 = np.maximum(16, ((maxcnt + 15) // 16) * 16)  # descriptor counts
    bkq = (n16 + P - 1) // P  # blocks per segment
    nblocks = int(bkq.sum())
    nstream = nblocks * P

    seg_off = np.zeros((NCHUNK, N_PASS), np.int64)  # row offsets
    off = 0
    for q in range(N_PASS):
        for k in range(NCHUNK):
            seg_off[k, q] = off
            off += int(bkq[k, q]) * P

    iota = np.tile(np.arange(P, dtype=np.float32)[None, :], (P, 1))

    in_maps = []
    for c in range(N_CORES):
        node_s, rows_s, q_s, k_s, msk_c = per_core[c]
        stream_idx = np.zeros(nstream, np.int16)
        stream_rel = np.full(nstream, 200.0, np.float32)
        seg_ids = q_s * NCHUNK + k_s
        change = np.r_[True, seg_ids[1:] != seg_ids[:-1]]
        seg_first = np.where(change)[0]
        within = np.arange(len(seg_ids)) - np.repeat(
            seg_first, np.diff(np.r_[seg_first, len(seg_ids)])
        )
        pos = seg_off[k_s, q_s] + within
        stream_idx[pos] = (rows_s - q_s * PASS_ROWS).astype(np.int16)
        stream_rel[pos] = (node_s - k_s * P).astype(np.float32)

        blk = stream_idx.reshape(nstream // 16, 16).T
        idx_sb = np.ascontiguousarray(np.tile(blk, (8, 1)))
        rel_sb = np.ascontiguousarray(stream_rel.reshape(nblocks, P).T)

        mpad = np.zeros((NPAD, S), np.float32)
        mpad[:NPC] = msk_c.astype(np.float32)
        msk_sb = np.ascontiguousarray(
            mpad.reshape(NCHUNK, P, S).transpose(1, 0, 2).reshape(P, NCHUNK * S)
        )

        in_maps.append(
            {
                "features": feats,
                "idx_sb": idx_sb,
                "rel_sb": rel_sb,
                "msk_sb": msk_sb,
                "iota_sb": iota,
            }
        )
    return bkq, n16, in_maps


_CACHE: dict[bytes, bass.Bass] = {}


def kernel(features, neighbor_idx, neighbor_mask, _trace=False):
    bkq, n16, in_maps = _marshal(features, neighbor_idx, neighbor_mask)
    key = bkq.tobytes() + n16.tobytes()
    nc = _CACHE.get(key)
    if nc is None:
        nc = build_program(bkq, n16)
        _CACHE[key] = nc
    res = bass_utils.run_bass_kernel_spmd(
        nc, in_maps, core_ids=list(range(N_CORES)), trace=_trace
    )
    outs = [r["out_sb"][:NPC] for r in res.results]
    if _trace:
        kernel.last_results = res
    return np.ascontiguousarray(np.concatenate(outs, axis=0), dtype=np.float32)


# revision 10
# speedup vs baseline: 1.0064x; 1.0064x over previous
"""v4: grouped big dma_gather instructions + 16-granular packed segments.

Stream per pass: (chunk,pass) segments packed back-to-back at 16-descriptor
granularity (n16 = max-over-cores ceil16 of pair count). Chunk boundaries may
fall inside 128-row blocks; rel-ids are stored as node_local mod 256 so
adjacent chunks occupy disjoint halves of a 256-wide iota, and a boundary
block issues one masked matmul per overlapped chunk. Gathers are issued as a
few big multi-block instructions per pass (Q7 fixed cost ~1.08us/instr).
"""

import numpy as np

import concourse.bacc as bacc
import concourse.bass as bass
import concourse.mybir as mybir
import concourse.tile as tile
from concourse import bass_utils

N_NODES = 100000
S = 16
D = 128
N_CORES = 8
NPC = N_NODES // N_CORES
P = 128
NCHUNK = 98
NPAD = NCHUNK * P
N_PASS = 4
PASS_ROWS = 25000
GB = 16  # max blocks per gather instruction

_f32 = mybir.dt.float32
_i16 = mybir.dt.int16

X = mybir.AxisListType.X
is_eq = mybir.AluOpType.is_equal


def _layout(n16):
    """Compute stream layout + block schedule from n16 [NCHUNK, N_PASS].

    Returns (nblocks, pass_blk0, pass_nblk, pass_desc, seg_off, sched) where
    sched[b] = list of (k, start, stop) for block b, and seg_off[k,q] is the
    row offset of segment (q,k) in the stream.
    """
    seg_off = np.zeros((NCHUNK, N_PASS), np.int64)
    pass_blk0 = []
    pass_nblk = []
    pass_desc = []
    blk = 0
    for q in range(N_PASS):
        pass_blk0.append(blk)
        row = blk * P
        for k in range(NCHUNK):
            seg_off[k, q] = row
            row += int(n16[k, q])
        ndesc = row - blk * P
        pass_desc.append(ndesc)
        nb = (ndesc + P - 1) // P
        pass_nblk.append(nb)
        blk += nb
    nblocks = blk

    sched = [[] for _ in range(nblocks)]
    for q in range(N_PASS):
        for k in range(NCHUNK):
            a = int(seg_off[k, q])
            z = a + int(n16[k, q])
            b0, b1 = a // P, (z - 1) // P
            for b in range(b0, b1 + 1):
                sched[b].append((k, b == b0, b == b1))
    # parity-collision check: no block may host two same-parity chunks
    for b, lst in enumerate(sched):
        ks = [k for k, _, _ in lst]
        assert len(set(k % 2 for k in ks)) == len(ks), (b, ks)
    return nblocks, pass_blk0, pass_nblk, pass_desc, seg_off, sched


def build_program(n16: np.ndarray) -> bass.Bass:
    nblocks, pass_blk0, pass_nblk, pass_desc, seg_off, sched = _layout(n16)
    nstream = nblocks * P

    nc = bacc.Bacc("TRN2", target_bir_lowering=False, debug=False)
    feat = nc.dram_tensor("features", [N_NODES, D], _f32, kind="ExternalInput").ap()
    idx_d = nc.dram_tensor("idx_sb", [P, nstream // 16], _i16, kind="ExternalInput").ap()
    rel_d = nc.dram_tensor("rel_sb", [P, nblocks], _f32, kind="ExternalInput").ap()
    msk_d = nc.dram_tensor("msk_sb", [P, NCHUNK * S], _f32, kind="ExternalInput").ap()
    iota_d = nc.dram_tensor("iota_sb", [P, 2 * P], _f32, kind="ExternalInput").ap()
    out_d = nc.dram_tensor("out_sb", [NPAD, D], _f32, kind="ExternalOutput").ap()

    with tile.TileContext(nc) as tc:
        with (
            tc.tile_pool(name="pre", bufs=1) as ppool,
            tc.tile_pool(name="seg", bufs=12) as segpool,
            tc.tile_pool(name="ob", bufs=3) as opool,
            tc.tile_pool(name="acc", bufs=1) as apool,
            tc.tile_pool(name="g", bufs=6) as gpool,
            tc.tile_pool(name="ps", bufs=6, space="PSUM") as pspool,
        ):
            idx_t = ppool.tile([P, nstream // 16], _i16)
            rel_t = ppool.tile([P, nblocks], _f32)
            msk_t = ppool.tile([P, NCHUNK * S], _f32)
            iota_t = ppool.tile([P, 2 * P], _f32)
            nc.sync.dma_start(out=idx_t[:], in_=idx_d[:, :])
            nc.sync.dma_start(out=rel_t[:], in_=rel_d[:, :])
            nc.sync.dma_start(out=msk_t[:], in_=msk_d[:, :])
            nc.sync.dma_start(out=iota_t[:], in_=iota_d[:, :])

            cnt = ppool.tile([P, NCHUNK], _f32)
            invc = ppool.tile([P, NCHUNK], _f32)
            nc.vector.reduce_sum(
                out=cnt[:], in_=msk_t[:].rearrange("p (k j) -> p k j", j=S), axis=X
            )
            nc.vector.tensor_scalar_max(out=invc[:], in0=cnt[:], scalar1=1.0)
            nc.vector.reciprocal(out=invc[:], in_=invc[:])

            acc = apool.tile([P, NCHUNK * D], _f32)

            # gather instructions + block->tile map
            gmap = {}
            for q in range(N_PASS):
                b0q = pass_blk0[q]
                nbq = pass_nblk[q]
                dq = pass_desc[q]
                g0 = 0
                while g0 < nbq:
                    g1 = min(g0 + GB, nbq)
                    ndesc = min(g1 * P, dq) - g0 * P
                    nb = g1 - g0
                    gbuf = gpool.tile([P, nb * D], _f32, tag="g")
                    nc.gpsimd.dma_gather(
                        out_ap=gbuf[:].rearrange("p (b d) -> p b d", d=D),
                        in_ap=feat[q * PASS_ROWS : (q + 1) * PASS_ROWS, :],
                        idxs_ap=idx_t[
                            :, (b0q + g0) * 8 : (b0q + g0) * 8 + ndesc // 16
                        ],
                        num_idxs=ndesc,
                        num_idxs_reg=ndesc,
                        elem_size=D,
                        single_packet=False,
                    )
                    for b in range(g0, g1):
                        gmap[b0q + b] = (gbuf, b - g0)
                    g0 = g1

            ps_live = {}
            for b in range(nblocks):
                gbuf, off = gmap[b]
                for k, st, sp in sched[b]:
                    seg_t = segpool.tile([P, P], _f32, tag="seg")
                    half = slice((k % 2) * P, (k % 2) * P + P)
                    nc.vector.tensor_scalar(
                        out=seg_t[:],
                        in0=iota_t[:, half],
                        scalar1=rel_t[:, b : b + 1],
                        scalar2=None,
                        op0=is_eq,
                    )
                    if st:
                        ps_new = pspool.tile([P, D], _f32, tag="ps", space="PSUM")
                        ps_live[k] = ps_new
                    nc.tensor.matmul(
                        out=ps_live[k][:],
                        lhsT=seg_t[:],
                        rhs=gbuf[:, off * D : (off + 1) * D],
                        start=st,
                        stop=sp,
                    )
                    if sp:
                        ps = ps_live.pop(k)
                        q = next(
                            qq for qq in range(N_PASS)
                            if pass_blk0[qq] <= b < pass_blk0[qq] + pass_nblk[qq]
                        )
                        if q == 0:
                            nc.vector.tensor_copy(
                                out=acc[:, k * D : (k + 1) * D], in_=ps[:]
                            )
                        else:
                            nc.vector.tensor_add(
                                out=acc[:, k * D : (k + 1) * D],
                                in0=acc[:, k * D : (k + 1) * D],
                                in1=ps[:],
                            )
                        if q == N_PASS - 1:
                            ob = opool.tile([P, D], _f32, tag="ob")
                            nc.vector.tensor_scalar_mul(
                                out=ob[:],
                                in0=acc[:, k * D : (k + 1) * D],
                                scalar1=invc[:, k : k + 1],
                            )
                            nc.sync.dma_start(
                                out=out_d[k * P : (k + 1) * P, :], in_=ob[:]
                            )
    nc.finalize()
    return nc


def _marshal(features, neighbor_idx, neighbor_mask):
    feats = np.ascontiguousarray(features, dtype=np.float32)
    msk = np.asarray(neighbor_mask, dtype=bool)
    idx = np.asarray(neighbor_idx, dtype=np.int64)

    per_core = []
    counts_all = np.zeros((N_CORES, NCHUNK, N_PASS), np.int64)
    for c in range(N_CORES):
        sl = slice(c * NPC, (c + 1) * NPC)
        idx_c = idx[sl]
        msk_c = msk[sl]
        node_l, j = np.nonzero(msk_c)
        rows = idx_c[node_l, j]
        q = rows // PASS_ROWS
        k = node_l // P
        order = np.lexsort((node_l, k, q))
        per_core.append(
            (node_l[order], rows[order], q[order], k[order], msk_c)
        )
        cnt_kq = np.zeros((NCHUNK, N_PASS), np.int64)
        np.add.at(cnt_kq, (k[order], q[order]), 1)
        counts_all[c] = cnt_kq

    maxcnt = counts_all.max(axis=0)
    n16 = np.maximum(16, ((maxcnt + 15) // 16) * 16)

    # _layout asserts no block hosts two same-parity chunks (true whenever
    # every segment is >~96 rows; holds by large margin for this workload)
    nblocks, pass_blk0, pass_nblk, pass_desc, seg_off, sched = _layout(n16)
    nstream = nblocks * P

    iota = np.tile(np.arange(2 * P, dtype=np.float32)[None, :], (P, 1))

    in_maps = []
    for c in range(N_CORES):
        node_s, rows_s, q_s, k_s, msk_c = per_core[c]
        stream_idx = np.zeros(nstream, np.int16)
        stream_rel = np.full(nstream, 300.0, np.float32)
        seg_ids = q_s * NCHUNK + k_s
        change = np.r_[True, seg_ids[1:] != seg_ids[:-1]]
        seg_first = np.where(change)[0]
        within = np.arange(len(seg_ids)) - np.repeat(
            seg_first, np.diff(np.r_[seg_first, len(seg_ids)])
        )
        pos = seg_off[k_s, q_s] + within
        stream_idx[pos] = (rows_s - q_s * PASS_ROWS).astype(np.int16)
        stream_rel[pos] = (node_s % 256).astype(np.float32)

        blk = stream_idx.reshape(nstream // 16, 16).T
        idx_sb = np.ascontiguousarray(np.tile(blk, (8, 1)))
        rel_sb = np.ascontiguousarray(stream_rel.reshape(nblocks, P).T)

        mpad = np.zeros((NPAD, S), np.float32)
        mpad[:NPC] = msk_c.astype(np.float32)
        msk_sb = np.ascontiguousarray(
            mpad.reshape(NCHUNK, P, S).transpose(1, 0, 2).reshape(P, NCHUNK * S)
        )
        in_maps.append(
            {
                "features": feats,
                "idx_sb": idx_sb,
                "rel_sb": rel_sb,
                "msk_sb": msk_sb,
                "iota_sb": iota,
            }
        )
    return n16, in_maps


_CACHE: dict[bytes, bass.Bass] = {}


def kernel(features, neighbor_idx, neighbor_mask, _trace=False):
    n16, in_maps = _marshal(features, neighbor_idx, neighbor_mask)
    key = n16.tobytes()
    nc = _CACHE.get(key)
    if nc is None:
        nc = build_program(n16)
        _CACHE[key] = nc
    res = bass_utils.run_bass_kernel_spmd(
        nc, in_maps, core_ids=list(range(N_CORES)), trace=_trace
    )
    outs = [r["out_sb"][:NPC] for r in res.results]
    if _trace:
        kernel.last_results = res
    return np.ascontiguousarray(np.concatenate(outs, axis=0), dtype=np.float32)


# revision 11
# speedup vs baseline: 1.1838x; 1.1763x over previous
"""v8: v6 + greedy pass-balanced node->chunk assignment (uniform segments).

Stream per pass: (chunk,pass) segments packed back-to-back at 16-descriptor
granularity (n16 = max-over-cores ceil16 of pair count). Chunk boundaries may
fall inside 128-row blocks; rel-ids are stored as node_local mod 256 so
adjacent chunks occupy disjoint halves of a 256-wide iota, and a boundary
block issues one masked matmul per overlapped chunk. Gathers are issued as a
few big multi-block instructions per pass (Q7 fixed cost ~1.08us/instr).
"""

import numpy as np

import concourse.bacc as bacc
import concourse.bass as bass
import concourse.mybir as mybir
import concourse.tile as tile
from concourse import bass_utils

N_NODES = 100000
S = 16
D = 128
N_CORES = 8
NPC = N_NODES // N_CORES
P = 128
NCHUNK = 98
NPAD = NCHUNK * P
N_PASS = 4
PASS_ROWS = 25000
GB = 16  # max blocks per gather instruction

_f32 = mybir.dt.float32
_i16 = mybir.dt.int16

X = mybir.AxisListType.X
is_eq = mybir.AluOpType.is_equal


def _layout(n16):
    """Compute stream layout + block schedule from n16 [NCHUNK, N_PASS].

    Returns (nblocks, pass_blk0, pass_nblk, pass_desc, seg_off, sched) where
    sched[b] = list of (k, start, stop) for block b, and seg_off[k,q] is the
    row offset of segment (q,k) in the stream.
    """
    seg_off = np.zeros((NCHUNK, N_PASS), np.int64)
    pass_blk0 = []
    pass_nblk = []
    pass_desc = []
    blk = 0
    for q in range(N_PASS):
        pass_blk0.append(blk)
        row = blk * P
        for k in range(NCHUNK):
            seg_off[k, q] = row
            row += int(n16[k, q])
        ndesc = row - blk * P
        pass_desc.append(ndesc)
        nb = (ndesc + P - 1) // P
        pass_nblk.append(nb)
        blk += nb
    nblocks = blk

    sched = [[] for _ in range(nblocks)]
    for q in range(N_PASS):
        for k in range(NCHUNK):
            a = int(seg_off[k, q])
            z = a + int(n16[k, q])
            b0, b1 = a // P, (z - 1) // P
            for b in range(b0, b1 + 1):
                sched[b].append((k, b == b0, b == b1))
    # parity-collision check: no block may host two same-parity chunks
    for b, lst in enumerate(sched):
        ks = [k for k, _, _ in lst]
        assert len(set(k % 2 for k in ks)) == len(ks), (b, ks)
    return nblocks, pass_blk0, pass_nblk, pass_desc, seg_off, sched


def build_program(n16: np.ndarray) -> bass.Bass:
    nblocks, pass_blk0, pass_nblk, pass_desc, seg_off, sched = _layout(n16)
    nstream = nblocks * P

    nc = bacc.Bacc("TRN2", target_bir_lowering=False, debug=False)
    feat = nc.dram_tensor("features", [N_NODES, D], _f32, kind="ExternalInput").ap()
    idx_d = nc.dram_tensor("idx_sb", [P, nstream // 16], _i16, kind="ExternalInput").ap()
    rel_d = nc.dram_tensor("rel_sb", [P, nblocks], _f32, kind="ExternalInput").ap()
    msk_d = nc.dram_tensor("msk_sb", [P, NCHUNK * S], _f32, kind="ExternalInput").ap()
    iota_d = nc.dram_tensor("iota_sb", [P, 2 * P], _f32, kind="ExternalInput").ap()
    out_d = nc.dram_tensor("out_sb", [NPAD, D], _f32, kind="ExternalOutput").ap()

    with tile.TileContext(nc) as tc:
        with (
            tc.tile_pool(name="pre", bufs=1) as ppool,
            tc.tile_pool(name="seg", bufs=12) as segpool,
            tc.tile_pool(name="ob", bufs=3) as opool,
            tc.tile_pool(name="acc", bufs=1) as apool,
            tc.tile_pool(name="g", bufs=8) as gpool,
            tc.tile_pool(name="ps", bufs=6, space="PSUM") as pspool,
        ):
            idx_t = ppool.tile([P, nstream // 16], _i16)
            rel_t = ppool.tile([P, nblocks], _f32)
            msk_t = ppool.tile([P, NCHUNK * S], _f32)
            iota_t = ppool.tile([P, 2 * P], _f32)
            nc.sync.dma_start(out=idx_t[:], in_=idx_d[:, :])
            nc.sync.dma_start(out=rel_t[:], in_=rel_d[:, :])
            nc.sync.dma_start(out=msk_t[:], in_=msk_d[:, :])
            nc.sync.dma_start(out=iota_t[:], in_=iota_d[:, :])

            cnt = ppool.tile([P, NCHUNK], _f32)
            invc = ppool.tile([P, NCHUNK], _f32)
            nc.vector.reduce_sum(
                out=cnt[:], in_=msk_t[:].rearrange("p (k j) -> p k j", j=S), axis=X
            )
            nc.vector.tensor_scalar_max(out=invc[:], in0=cnt[:], scalar1=1.0)
            nc.vector.reciprocal(out=invc[:], in_=invc[:])

            acc = apool.tile([P, NCHUNK * D], _f32)

            # gather instructions + block->tile map
            gmap = {}
            for q in range(N_PASS):
                b0q = pass_blk0[q]
                nbq = pass_nblk[q]
                dq = pass_desc[q]
                g0 = 0
                while g0 < nbq:
                    g1 = min(g0 + GB, nbq)
                    ndesc = min(g1 * P, dq) - g0 * P
                    nb = g1 - g0
                    gbuf = gpool.tile([P, nb * D], _f32, tag="g")
                    nc.gpsimd.dma_gather(
                        out_ap=gbuf[:].rearrange("p (b d) -> p b d", d=D),
                        in_ap=feat[q * PASS_ROWS : (q + 1) * PASS_ROWS, :],
                        idxs_ap=idx_t[
                            :, (b0q + g0) * 8 : (b0q + g0) * 8 + ndesc // 16
                        ],
                        num_idxs=ndesc,
                        num_idxs_reg=ndesc,
                        elem_size=D,
                        single_packet=False,
                    )
                    for b in range(g0, g1):
                        gmap[b0q + b] = (gbuf, b - g0)
                    g0 = g1

            ps_live = {}
            for b in range(nblocks):
                gbuf, off = gmap[b]
                for k, st, sp in sched[b]:
                    seg_t = segpool.tile([P, P], _f32, tag="seg")
                    half = slice((k % 2) * P, (k % 2) * P + P)
                    nc.vector.tensor_scalar(
                        out=seg_t[:],
                        in0=iota_t[:, half],
                        scalar1=rel_t[:, b : b + 1],
                        scalar2=None,
                        op0=is_eq,
                    )
                    if st:
                        ps_new = pspool.tile([P, D], _f32, tag="ps", space="PSUM")
                        ps_live[k] = ps_new
                    nc.tensor.matmul(
                        out=ps_live[k][:],
                        lhsT=seg_t[:],
                        rhs=gbuf[:, off * D : (off + 1) * D],
                        start=st,
                        stop=sp,
                    )
                    if sp:
                        ps = ps_live.pop(k)
                        q = next(
                            qq for qq in range(N_PASS)
                            if pass_blk0[qq] <= b < pass_blk0[qq] + pass_nblk[qq]
                        )
                        if q == 0:
                            nc.vector.tensor_copy(
                                out=acc[:, k * D : (k + 1) * D], in_=ps[:]
                            )
                        else:
                            nc.vector.tensor_add(
                                out=acc[:, k * D : (k + 1) * D],
                                in0=acc[:, k * D : (k + 1) * D],
                                in1=ps[:],
                            )
                        if q == N_PASS - 1:
                            ob = opool.tile([P, D], _f32, tag="ob")
                            nc.vector.tensor_scalar_mul(
                                out=ob[:],
                                in0=acc[:, k * D : (k + 1) * D],
                                scalar1=invc[:, k : k + 1],
                            )
                            nc.sync.dma_start(
                                out=out_d[k * P : (k + 1) * P, :], in_=ob[:]
                            )
    nc.finalize()
    return nc


def _greedy_bins(c4):
    """Assign NPC nodes to NCHUNK bins (<=P each), balancing all pass sums."""
    order = np.argsort(-c4.max(1), kind="stable")
    sums = np.zeros((NCHUNK, N_PASS), np.int64)
    fill = np.zeros(NCHUNK, np.int64)
    bins = np.empty(NPC, np.int64)
    for n in order:
        cand = fill < P
        m = np.where(cand[:, None], sums + c4[n], 1 << 40).max(1)
        b = int(np.argmin(m))
        bins[n] = b
        sums[b] += c4[n]
        fill[b] += 1
    return bins


def _marshal(features, neighbor_idx, neighbor_mask):
    feats = np.ascontiguousarray(features, dtype=np.float32)
    msk = np.asarray(neighbor_mask, dtype=bool)
    idx = np.asarray(neighbor_idx, dtype=np.int64)

    per_core = []
    counts_all = np.zeros((N_CORES, NCHUNK, N_PASS), np.int64)
    for c in range(N_CORES):
        sl = slice(c * NPC, (c + 1) * NPC)
        idx_c = idx[sl]
        msk_c = msk[sl]
        qn = idx_c // PASS_ROWS
        c4 = np.stack([((msk_c) & (qn == qq)).sum(1) for qq in range(N_PASS)], 1)
        bins = _greedy_bins(c4)
        # position of each node within its bin
        border = np.lexsort((np.arange(NPC), bins))
        pos = np.empty(NPC, np.int64)
        boff = np.zeros(NCHUNK, np.int64)
        np.add.at(boff, bins, 1)
        starts = np.r_[0, np.cumsum(boff)[:-1]]
        pos[border] = np.arange(NPC) - np.repeat(starts, boff)
        # inverse map: output row k*P+pos -> node
        inv = bins * P + pos  # node -> out row

        node_l, j = np.nonzero(msk_c)
        rows = idx_c[node_l, j]
        q = rows // PASS_ROWS
        k = bins[node_l]
        order = np.lexsort((node_l, k, q))
        per_core.append(
            (node_l[order], rows[order], q[order], k[order], msk_c, bins, pos, inv)
        )
        cnt_kq = np.zeros((NCHUNK, N_PASS), np.int64)
        np.add.at(cnt_kq, (k[order], q[order]), 1)
        counts_all[c] = cnt_kq

    maxcnt = counts_all.max(axis=0)
    n16 = np.maximum(16, ((maxcnt + 15) // 16) * 16)

    # _layout asserts no block hosts two same-parity chunks (true whenever
    # every segment is >~96 rows; holds by large margin for this workload)
    nblocks, pass_blk0, pass_nblk, pass_desc, seg_off, sched = _layout(n16)
    nstream = nblocks * P

    iota = np.tile(np.arange(2 * P, dtype=np.float32)[None, :], (P, 1))

    in_maps = []
    invs = []
    for c in range(N_CORES):
        node_s, rows_s, q_s, k_s, msk_c, bins, posn, inv = per_core[c]
        stream_idx = np.zeros(nstream, np.int16)
        stream_rel = np.full(nstream, 300.0, np.float32)
        seg_ids = q_s * NCHUNK + k_s
        change = np.r_[True, seg_ids[1:] != seg_ids[:-1]]
        seg_first = np.where(change)[0]
        within = np.arange(len(seg_ids)) - np.repeat(
            seg_first, np.diff(np.r_[seg_first, len(seg_ids)])
        )
        spos = seg_off[k_s, q_s] + within
        stream_idx[spos] = (rows_s - q_s * PASS_ROWS).astype(np.int16)
        stream_rel[spos] = ((k_s % 2) * P + posn[node_s]).astype(np.float32)

        blk = stream_idx.reshape(nstream // 16, 16).T
        idx_sb = np.ascontiguousarray(np.tile(blk, (8, 1)))
        rel_sb = np.ascontiguousarray(stream_rel.reshape(nblocks, P).T)

        # mask arranged by (chunk, pos): slot (k, r) -> node with inv==k*P+r
        mpad = np.zeros((NPAD, S), np.float32)
        mpad[inv] = msk_c.astype(np.float32)
        msk_sb = np.ascontiguousarray(
            mpad.reshape(NCHUNK, P, S).transpose(1, 0, 2).reshape(P, NCHUNK * S)
        )
        in_maps.append(
            {
                "features": feats,
                "idx_sb": idx_sb,
                "rel_sb": rel_sb,
                "msk_sb": msk_sb,
                "iota_sb": iota,
            }
        )
        invs.append(inv)
    return n16, in_maps, invs


_CACHE: dict[bytes, bass.Bass] = {}


def kernel(features, neighbor_idx, neighbor_mask, _trace=False):
    n16, in_maps, invs = _marshal(features, neighbor_idx, neighbor_mask)
    key = n16.tobytes()
    nc = _CACHE.get(key)
    if nc is None:
        nc = build_program(n16)
        _CACHE[key] = nc
    res = bass_utils.run_bass_kernel_spmd(
        nc, in_maps, core_ids=list(range(N_CORES)), trace=_trace
    )
    outs = [r["out_sb"][invs[c]] for c, r in enumerate(res.results)]
    if _trace:
        kernel.last_results = res
    return np.ascontiguousarray(np.concatenate(outs, axis=0), dtype=np.float32)


# revision 12
# speedup vs baseline: 1.1858x; 1.0017x over previous
"""v8: v6 + greedy pass-balanced node->chunk assignment (uniform segments).

Stream per pass: (chunk,pass) segments packed back-to-back at 16-descriptor
granularity (n16 = max-over-cores ceil16 of pair count). Chunk boundaries may
fall inside 128-row blocks; rel-ids are stored as node_local mod 256 so
adjacent chunks occupy disjoint halves of a 256-wide iota, and a boundary
block issues one masked matmul per overlapped chunk. Gathers are issued as a
few big multi-block instructions per pass (Q7 fixed cost ~1.08us/instr).
"""

import numpy as np

import concourse.bacc as bacc
import concourse.bass as bass
import concourse.mybir as mybir
import concourse.tile as tile
from concourse import bass_utils

N_NODES = 100000
S = 16
D = 128
N_CORES = 8
NPC = N_NODES // N_CORES
P = 128
NCHUNK = 98
NPAD = NCHUNK * P
N_PASS = 4
PASS_ROWS = 25000
GB = 16  # max blocks per gather instruction

_f32 = mybir.dt.float32
_i16 = mybir.dt.int16

X = mybir.AxisListType.X
is_eq = mybir.AluOpType.is_equal


def _layout(n16):
    """Compute stream layout + block schedule from n16 [NCHUNK, N_PASS].

    Returns (nblocks, pass_blk0, pass_nblk, pass_desc, seg_off, sched) where
    sched[b] = list of (k, start, stop) for block b, and seg_off[k,q] is the
    row offset of segment (q,k) in the stream.
    """
    seg_off = np.zeros((NCHUNK, N_PASS), np.int64)
    pass_blk0 = []
    pass_nblk = []
    pass_desc = []
    blk = 0
    for q in range(N_PASS):
        pass_blk0.append(blk)
        row = blk * P
        for k in range(NCHUNK):
            seg_off[k, q] = row
            row += int(n16[k, q])
        ndesc = row - blk * P
        pass_desc.append(ndesc)
        nb = (ndesc + P - 1) // P
        pass_nblk.append(nb)
        blk += nb
    nblocks = blk

    sched = [[] for _ in range(nblocks)]
    for q in range(N_PASS):
        for k in range(NCHUNK):
            a = int(seg_off[k, q])
            z = a + int(n16[k, q])
            b0, b1 = a // P, (z - 1) // P
            for b in range(b0, b1 + 1):
                sched[b].append((k, b == b0, b == b1))
    # parity-collision check: no block may host two same-parity chunks
    for b, lst in enumerate(sched):
        ks = [k for k, _, _ in lst]
        assert len(set(k % 2 for k in ks)) == len(ks), (b, ks)
    return nblocks, pass_blk0, pass_nblk, pass_desc, seg_off, sched


def build_program(n16: np.ndarray) -> bass.Bass:
    nblocks, pass_blk0, pass_nblk, pass_desc, seg_off, sched = _layout(n16)
    nstream = nblocks * P

    nc = bacc.Bacc("TRN2", target_bir_lowering=False, debug=False)
    feat = nc.dram_tensor("features", [N_NODES, D], _f32, kind="ExternalInput").ap()
    idx_d = nc.dram_tensor("idx_sb", [P, nstream // 16], _i16, kind="ExternalInput").ap()
    rel_d = nc.dram_tensor("rel_sb", [P, nblocks], _f32, kind="ExternalInput").ap()
    msk_d = nc.dram_tensor("msk_sb", [P, NCHUNK * S], _f32, kind="ExternalInput").ap()
    iota_d = nc.dram_tensor("iota_sb", [P, 2 * P], _f32, kind="ExternalInput").ap()
    out_d = nc.dram_tensor("out_sb", [NPAD, D], _f32, kind="ExternalOutput").ap()

    with tile.TileContext(nc) as tc:
        with (
            tc.tile_pool(name="pre", bufs=1) as ppool,
            tc.tile_pool(name="seg", bufs=16) as segpool,
            tc.tile_pool(name="ob", bufs=3) as opool,
            tc.tile_pool(name="acc", bufs=1) as apool,
            tc.tile_pool(name="g", bufs=8) as gpool,
            tc.tile_pool(name="ps", bufs=8, space="PSUM") as pspool,
        ):
            idx_t = ppool.tile([P, nstream // 16], _i16)
            rel_t = ppool.tile([P, nblocks], _f32)
            msk_t = ppool.tile([P, NCHUNK * S], _f32)
            iota_t = ppool.tile([P, 2 * P], _f32)
            nc.sync.dma_start(out=idx_t[:], in_=idx_d[:, :])
            nc.sync.dma_start(out=rel_t[:], in_=rel_d[:, :])
            nc.sync.dma_start(out=msk_t[:], in_=msk_d[:, :])
            nc.sync.dma_start(out=iota_t[:], in_=iota_d[:, :])

            cnt = ppool.tile([P, NCHUNK], _f32)
            invc = ppool.tile([P, NCHUNK], _f32)
            nc.vector.reduce_sum(
                out=cnt[:], in_=msk_t[:].rearrange("p (k j) -> p k j", j=S), axis=X
            )
            nc.vector.tensor_scalar_max(out=invc[:], in0=cnt[:], scalar1=1.0)
            nc.vector.reciprocal(out=invc[:], in_=invc[:])

            acc = apool.tile([P, NCHUNK * D], _f32)

            # gather instructions + block->tile map
            gmap = {}
            for q in range(N_PASS):
                b0q = pass_blk0[q]
                nbq = pass_nblk[q]
                dq = pass_desc[q]
                g0 = 0
                while g0 < nbq:
                    g1 = min(g0 + GB, nbq)
                    ndesc = min(g1 * P, dq) - g0 * P
                    nb = g1 - g0
                    gbuf = gpool.tile([P, nb * D], _f32, tag="g")
                    nc.gpsimd.dma_gather(
                        out_ap=gbuf[:].rearrange("p (b d) -> p b d", d=D),
                        in_ap=feat[q * PASS_ROWS : (q + 1) * PASS_ROWS, :],
                        idxs_ap=idx_t[
                            :, (b0q + g0) * 8 : (b0q + g0) * 8 + ndesc // 16
                        ],
                        num_idxs=ndesc,
                        num_idxs_reg=ndesc,
                        elem_size=D,
                        single_packet=False,
                    )
                    for b in range(g0, g1):
                        gmap[b0q + b] = (gbuf, b - g0)
                    g0 = g1

            ps_live = {}
            for b in range(nblocks):
                gbuf, off = gmap[b]
                for k, st, sp in sched[b]:
                    seg_t = segpool.tile([P, P], _f32, tag="seg")
                    half = slice((k % 2) * P, (k % 2) * P + P)
                    nc.vector.tensor_scalar(
                        out=seg_t[:],
                        in0=iota_t[:, half],
                        scalar1=rel_t[:, b : b + 1],
                        scalar2=None,
                        op0=is_eq,
                    )
                    if st:
                        ps_new = pspool.tile([P, D], _f32, tag="ps", space="PSUM")
                        ps_live[k] = ps_new
                    nc.tensor.matmul(
                        out=ps_live[k][:],
                        lhsT=seg_t[:],
                        rhs=gbuf[:, off * D : (off + 1) * D],
                        start=st,
                        stop=sp,
                    )
                    if sp:
                        ps = ps_live.pop(k)
                        q = next(
                            qq for qq in range(N_PASS)
                            if pass_blk0[qq] <= b < pass_blk0[qq] + pass_nblk[qq]
                        )
                        if q == 0:
                            nc.scalar.activation(
                                out=acc[:, k * D : (k + 1) * D],
                                in_=ps[:],
                                func=mybir.ActivationFunctionType.Copy,
                            )
                        else:
                            nc.vector.tensor_add(
                                out=acc[:, k * D : (k + 1) * D],
                                in0=acc[:, k * D : (k + 1) * D],
                                in1=ps[:],
                            )
                        if q == N_PASS - 1:
                            ob = opool.tile([P, D], _f32, tag="ob")
                            nc.scalar.activation(
                                out=ob[:],
                                in_=acc[:, k * D : (k + 1) * D],
                                func=mybir.ActivationFunctionType.Copy,
                                scale=invc[:, k : k + 1],
                            )
                            nc.sync.dma_start(
                                out=out_d[k * P : (k + 1) * P, :], in_=ob[:]
                            )
    nc.finalize()
    return nc


def _greedy_bins(c4):
    """Assign NPC nodes to NCHUNK bins (<=P each), balancing all pass sums."""
    order = np.argsort(-c4.max(1), kind="stable")
    sums = np.zeros((NCHUNK, N_PASS), np.int64)
    fill = np.zeros(NCHUNK, np.int64)
    bins = np.empty(NPC, np.int64)
    for n in order:
        cand = fill < P
        m = np.where(cand[:, None], sums + c4[n], 1 << 40).max(1)
        b = int(np.argmin(m))
        bins[n] = b
        sums[b] += c4[n]
        fill[b] += 1
    return bins


def _marshal(features, neighbor_idx, neighbor_mask):
    feats = np.ascontiguousarray(features, dtype=np.float32)
    msk = np.asarray(neighbor_mask, dtype=bool)
    idx = np.asarray(neighbor_idx, dtype=np.int64)

    per_core = []
    counts_all = np.zeros((N_CORES, NCHUNK, N_PASS), np.int64)
    for c in range(N_CORES):
        sl = slice(c * NPC, (c + 1) * NPC)
        idx_c = idx[sl]
        msk_c = msk[sl]
        qn = idx_c // PASS_ROWS
        c4 = np.stack([((msk_c) & (qn == qq)).sum(1) for qq in range(N_PASS)], 1)
        bins = _greedy_bins(c4)
        # position of each node within its bin
        border = np.lexsort((np.arange(NPC), bins))
        pos = np.empty(NPC, np.int64)
        boff = np.zeros(NCHUNK, np.int64)
        np.add.at(boff, bins, 1)
        starts = np.r_[0, np.cumsum(boff)[:-1]]
        pos[border] = np.arange(NPC) - np.repeat(starts, boff)
        # inverse map: output row k*P+pos -> node
        inv = bins * P + pos  # node -> out row

        node_l, j = np.nonzero(msk_c)
        rows = idx_c[node_l, j]
        q = rows // PASS_ROWS
        k = bins[node_l]
        order = np.lexsort((node_l, k, q))
        per_core.append(
            (node_l[order], rows[order], q[order], k[order], msk_c, bins, pos, inv)
        )
        cnt_kq = np.zeros((NCHUNK, N_PASS), np.int64)
        np.add.at(cnt_kq, (k[order], q[order]), 1)
        counts_all[c] = cnt_kq

    maxcnt = counts_all.max(axis=0)
    n16 = np.maximum(16, ((maxcnt + 15) // 16) * 16)

    # _layout asserts no block hosts two same-parity chunks (true whenever
    # every segment is >~96 rows; holds by large margin for this workload)
    nblocks, pass_blk0, pass_nblk, pass_desc, seg_off, sched = _layout(n16)
    nstream = nblocks * P

    iota = np.tile(np.arange(2 * P, dtype=np.float32)[None, :], (P, 1))

    in_maps = []
    invs = []
    for c in range(N_CORES):
        node_s, rows_s, q_s, k_s, msk_c, bins, posn, inv = per_core[c]
        stream_idx = np.zeros(nstream, np.int16)
        stream_rel = np.full(nstream, 300.0, np.float32)
        seg_ids = q_s * NCHUNK + k_s
        change = np.r_[True, seg_ids[1:] != seg_ids[:-1]]
        seg_first = np.where(change)[0]
        within = np.arange(len(seg_ids)) - np.repeat(
            seg_first, np.diff(np.r_[seg_first, len(seg_ids)])
        )
        spos = seg_off[k_s, q_s] + within
        stream_idx[spos] = (rows_s - q_s * PASS_ROWS).astype(np.int16)
        stream_rel[spos] = ((k_s % 2) * P + posn[node_s]).astype(np.float32)

        blk = stream_idx.reshape(nstream // 16, 16).T
        idx_sb = np.ascontiguousarray(np.tile(blk, (8, 1)))
        rel_sb = np.ascontiguousarray(stream_rel.reshape(nblocks, P).T)

        # mask arranged by (chunk, pos): slot (k, r) -> node with inv==k*P+r
        mpad = np.zeros((NPAD, S), np.float32)
        mpad[inv] = msk_c.astype(np.float32)
        msk_sb = np.ascontiguousarray(
            mpad.reshape(NCHUNK, P, S).transpose(1, 0, 2).reshape(P, NCHUNK * S)
        )
        in_maps.append(
            {
                "features": feats,
                "idx_sb": idx_sb,
                "rel_sb": rel_sb,
                "msk_sb": msk_sb,
                "iota_sb": iota,
            }
        )
        invs.append(inv)
    return n16, in_maps, invs


_CACHE: dict[bytes, bass.Bass] = {}


def kernel(features, neighbor_idx, neighbor_mask, _trace=False):
    n16, in_maps, invs = _marshal(features, neighbor_idx, neighbor_mask)
    key = n16.tobytes()
    nc = _CACHE.get(key)
    if nc is None:
        nc = build_program(n16)
        _CACHE[key] = nc
    res = bass_utils.run_bass_kernel_spmd(
        nc, in_maps, core_ids=list(range(N_CORES)), trace=_trace
    )
    outs = [r["out_sb"][invs[c]] for c, r in enumerate(res.results)]
    if _trace:
        kernel.last_results = res
    return np.ascontiguousarray(np.concatenate(outs, axis=0), dtype=np.float32)


# revision 13
# speedup vs baseline: 1.1912x; 1.0045x over previous
"""v8: v6 + greedy pass-balanced node->chunk assignment (uniform segments).

Stream per pass: (chunk,pass) segments packed back-to-back at 16-descriptor
granularity (n16 = max-over-cores ceil16 of pair count). Chunk boundaries may
fall inside 128-row blocks; rel-ids are stored as node_local mod 256 so
adjacent chunks occupy disjoint halves of a 256-wide iota, and a boundary
block issues one masked matmul per overlapped chunk. Gathers are issued as a
few big multi-block instructions per pass (Q7 fixed cost ~1.08us/instr).
"""

import numpy as np

import concourse.bacc as bacc
import concourse.bass as bass
import concourse.mybir as mybir
import concourse.tile as tile
from concourse import bass_utils

N_NODES = 100000
S = 16
D = 128
N_CORES = 8
NPC = N_NODES // N_CORES
P = 128
NCHUNK = 98
NPAD = NCHUNK * P
N_PASS = 4
PASS_ROWS = 25000
GB = 16  # max blocks per gather instruction

_f32 = mybir.dt.float32
_i16 = mybir.dt.int16

X = mybir.AxisListType.X
is_eq = mybir.AluOpType.is_equal


def _layout(n16):
    """Compute stream layout + block schedule from n16 [NCHUNK, N_PASS].

    Returns (nblocks, pass_blk0, pass_nblk, pass_desc, seg_off, sched) where
    sched[b] = list of (k, start, stop) for block b, and seg_off[k,q] is the
    row offset of segment (q,k) in the stream.
    """
    seg_off = np.zeros((NCHUNK, N_PASS), np.int64)
    pass_blk0 = []
    pass_nblk = []
    pass_desc = []
    blk = 0
    for q in range(N_PASS):
        pass_blk0.append(blk)
        row = blk * P
        for k in range(NCHUNK):
            seg_off[k, q] = row
            row += int(n16[k, q])
        ndesc = row - blk * P
        pass_desc.append(ndesc)
        nb = (ndesc + P - 1) // P
        pass_nblk.append(nb)
        blk += nb
    nblocks = blk

    sched = [[] for _ in range(nblocks)]
    for q in range(N_PASS):
        for k in range(NCHUNK):
            a = int(seg_off[k, q])
            z = a + int(n16[k, q])
            b0, b1 = a // P, (z - 1) // P
            for b in range(b0, b1 + 1):
                sched[b].append((k, b == b0, b == b1))
    # parity-collision check: no block may host two same-parity chunks
    for b, lst in enumerate(sched):
        ks = [k for k, _, _ in lst]
        assert len(set(k % 2 for k in ks)) == len(ks), (b, ks)
    return nblocks, pass_blk0, pass_nblk, pass_desc, seg_off, sched


def build_program(n16: np.ndarray) -> bass.Bass:
    nblocks, pass_blk0, pass_nblk, pass_desc, seg_off, sched = _layout(n16)
    nstream = nblocks * P

    nc = bacc.Bacc("TRN2", target_bir_lowering=False, debug=False)
    feat = nc.dram_tensor("features", [N_NODES, D], _f32, kind="ExternalInput").ap()
    idx_d = nc.dram_tensor("idx_sb", [P, nstream // 16], _i16, kind="ExternalInput").ap()
    rel_d = nc.dram_tensor("rel_sb", [P, nblocks], _f32, kind="ExternalInput").ap()
    msk_d = nc.dram_tensor("msk_sb", [P, NCHUNK * S], _f32, kind="ExternalInput").ap()
    iota_d = nc.dram_tensor("iota_sb", [P, 2 * P], _f32, kind="ExternalInput").ap()
    out_d = nc.dram_tensor("out_sb", [NPAD, D], _f32, kind="ExternalOutput").ap()

    with tile.TileContext(nc) as tc:
        with (
            tc.tile_pool(name="pre", bufs=1) as ppool,
            tc.tile_pool(name="seg", bufs=40) as segpool,
            tc.tile_pool(name="ob", bufs=3) as opool,
            tc.tile_pool(name="acc", bufs=1) as apool,
            tc.tile_pool(name="g", bufs=8) as gpool,
            tc.tile_pool(name="ps", bufs=8, space="PSUM") as pspool,
        ):
            idx_t = ppool.tile([P, nstream // 16], _i16)
            rel_t = ppool.tile([P, nblocks], _f32)
            msk_t = ppool.tile([P, NCHUNK * S], _f32)
            iota_t = ppool.tile([P, 2 * P], _f32)
            nc.sync.dma_start(out=idx_t[:], in_=idx_d[:, :])
            nc.sync.dma_start(out=rel_t[:], in_=rel_d[:, :])
            nc.sync.dma_start(out=msk_t[:], in_=msk_d[:, :])
            nc.sync.dma_start(out=iota_t[:], in_=iota_d[:, :])

            cnt = ppool.tile([P, NCHUNK], _f32)
            invc = ppool.tile([P, NCHUNK], _f32)
            nc.vector.reduce_sum(
                out=cnt[:], in_=msk_t[:].rearrange("p (k j) -> p k j", j=S), axis=X
            )
            nc.vector.tensor_scalar_max(out=invc[:], in0=cnt[:], scalar1=1.0)
            nc.vector.reciprocal(out=invc[:], in_=invc[:])

            acc = apool.tile([P, NCHUNK * D], _f32)

            # gather instructions + block->tile map
            gmap = {}
            for q in range(N_PASS):
                b0q = pass_blk0[q]
                nbq = pass_nblk[q]
                dq = pass_desc[q]
                g0 = 0
                while g0 < nbq:
                    g1 = min(g0 + GB, nbq)
                    ndesc = min(g1 * P, dq) - g0 * P
                    nb = g1 - g0
                    gbuf = gpool.tile([P, nb * D], _f32, tag="g")
                    nc.gpsimd.dma_gather(
                        out_ap=gbuf[:].rearrange("p (b d) -> p b d", d=D),
                        in_ap=feat[q * PASS_ROWS : (q + 1) * PASS_ROWS, :],
                        idxs_ap=idx_t[
                            :, (b0q + g0) * 8 : (b0q + g0) * 8 + ndesc // 16
                        ],
                        num_idxs=ndesc,
                        num_idxs_reg=ndesc,
                        elem_size=D,
                        single_packet=False,
                    )
                    for b in range(g0, g1):
                        gmap[b0q + b] = (gbuf, b - g0)
                    g0 = g1

            ps_live = {}
            for b in range(nblocks):
                gbuf, off = gmap[b]
                for k, st, sp in sched[b]:
                    seg_t = segpool.tile([P, P], _f32, tag="seg")
                    half = slice((k % 2) * P, (k % 2) * P + P)
                    nc.vector.tensor_scalar(
                        out=seg_t[:],
                        in0=iota_t[:, half],
                        scalar1=rel_t[:, b : b + 1],
                        scalar2=None,
                        op0=is_eq,
                    )
                    if st:
                        ps_new = pspool.tile([P, D], _f32, tag="ps", space="PSUM")
                        ps_live[k] = ps_new
                    nc.tensor.matmul(
                        out=ps_live[k][:],
                        lhsT=seg_t[:],
                        rhs=gbuf[:, off * D : (off + 1) * D],
                        start=st,
                        stop=sp,
                    )
                    if sp:
                        ps = ps_live.pop(k)
                        q = next(
                            qq for qq in range(N_PASS)
                            if pass_blk0[qq] <= b < pass_blk0[qq] + pass_nblk[qq]
                        )
                        if q == 0:
                            nc.scalar.activation(
                                out=acc[:, k * D : (k + 1) * D],
                                in_=ps[:],
                                func=mybir.ActivationFunctionType.Copy,
                            )
                        else:
                            nc.vector.tensor_add(
                                out=acc[:, k * D : (k + 1) * D],
                                in0=acc[:, k * D : (k + 1) * D],
                                in1=ps[:],
                            )
                        if q == N_PASS - 1:
                            ob = opool.tile([P, D], _f32, tag="ob")
                            nc.scalar.activation(
                                out=ob[:],
                                in_=acc[:, k * D : (k + 1) * D],
                                func=mybir.ActivationFunctionType.Copy,
                                scale=invc[:, k : k + 1],
                            )
                            nc.sync.dma_start(
                                out=out_d[k * P : (k + 1) * P, :], in_=ob[:]
                            )
    nc.finalize()
    return nc


def _greedy_bins(c4):
    """Assign NPC nodes to NCHUNK bins (<=P each), balancing all pass sums."""
    order = np.argsort(-c4.max(1), kind="stable")
    sums = np.zeros((NCHUNK, N_PASS), np.int64)
    fill = np.zeros(NCHUNK, np.int64)
    bins = np.empty(NPC, np.int64)
    for n in order:
        cand = fill < P
        m = np.where(cand[:, None], sums + c4[n], 1 << 40).max(1)
        b = int(np.argmin(m))
        bins[n] = b
        sums[b] += c4[n]
        fill[b] += 1
    return bins


def _marshal(features, neighbor_idx, neighbor_mask):
    feats = np.ascontiguousarray(features, dtype=np.float32)
    msk = np.asarray(neighbor_mask, dtype=bool)
    idx = np.asarray(neighbor_idx, dtype=np.int64)

    per_core = []
    counts_all = np.zeros((N_CORES, NCHUNK, N_PASS), np.int64)
    for c in range(N_CORES):
        sl = slice(c * NPC, (c + 1) * NPC)
        idx_c = idx[sl]
        msk_c = msk[sl]
        qn = idx_c // PASS_ROWS
        c4 = np.stack([((msk_c) & (qn == qq)).sum(1) for qq in range(N_PASS)], 1)
        bins = _greedy_bins(c4)
        # position of each node within its bin
        border = np.lexsort((np.arange(NPC), bins))
        pos = np.empty(NPC, np.int64)
        boff = np.zeros(NCHUNK, np.int64)
        np.add.at(boff, bins, 1)
        starts = np.r_[0, np.cumsum(boff)[:-1]]
        pos[border] = np.arange(NPC) - np.repeat(starts, boff)
        # inverse map: output row k*P+pos -> node
        inv = bins * P + pos  # node -> out row

        node_l, j = np.nonzero(msk_c)
        rows = idx_c[node_l, j]
        q = rows // PASS_ROWS
        k = bins[node_l]
        order = np.lexsort((node_l, k, q))
        per_core.append(
            (node_l[order], rows[order], q[order], k[order], msk_c, bins, pos, inv)
        )
        cnt_kq = np.zeros((NCHUNK, N_PASS), np.int64)
        np.add.at(cnt_kq, (k[order], q[order]), 1)
        counts_all[c] = cnt_kq

    maxcnt = counts_all.max(axis=0)
    n16 = np.maximum(16, ((maxcnt + 15) // 16) * 16)

    # _layout asserts no block hosts two same-parity chunks (true whenever
    # every segment is >~96 rows; holds by large margin for this workload)
    nblocks, pass_blk0, pass_nblk, pass_desc, seg_off, sched = _layout(n16)
    nstream = nblocks * P

    iota = np.tile(np.arange(2 * P, dtype=np.float32)[None, :], (P, 1))

    in_maps = []
    invs = []
    for c in range(N_CORES):
        node_s, rows_s, q_s, k_s, msk_c, bins, posn, inv = per_core[c]
        stream_idx = np.zeros(nstream, np.int16)
        stream_rel = np.full(nstream, 300.0, np.float32)
        seg_ids = q_s * NCHUNK + k_s
        change = np.r_[True, seg_ids[1:] != seg_ids[:-1]]
        seg_first = np.where(change)[0]
        within = np.arange(len(seg_ids)) - np.repeat(
            seg_first, np.diff(np.r_[seg_first, len(seg_ids)])
        )
        spos = seg_off[k_s, q_s] + within
        stream_idx[spos] = (rows_s - q_s * PASS_ROWS).astype(np.int16)
        stream_rel[spos] = ((k_s % 2) * P + posn[node_s]).astype(np.float32)

        blk = stream_idx.reshape(nstream // 16, 16).T
        idx_sb = np.ascontiguousarray(np.tile(blk, (8, 1)))
        rel_sb = np.ascontiguousarray(stream_rel.reshape(nblocks, P).T)

        # mask arranged by (chunk, pos): slot (k, r) -> node with inv==k*P+r
        mpad = np.zeros((NPAD, S), np.float32)
        mpad[inv] = msk_c.astype(np.float32)
        msk_sb = np.ascontiguousarray(
            mpad.reshape(NCHUNK, P, S).transpose(1, 0, 2).reshape(P, NCHUNK * S)
        )
        in_maps.append(
            {
                "features": feats,
                "idx_sb": idx_sb,
                "rel_sb": rel_sb,
                "msk_sb": msk_sb,
                "iota_sb": iota,
            }
        )
        invs.append(inv)
    return n16, in_maps, invs


_CACHE: dict[bytes, bass.Bass] = {}


def kernel(features, neighbor_idx, neighbor_mask, _trace=False):
    n16, in_maps, invs = _marshal(features, neighbor_idx, neighbor_mask)
    key = n16.tobytes()
    nc = _CACHE.get(key)
    if nc is None:
        nc = build_program(n16)
        _CACHE[key] = nc
    res = bass_utils.run_bass_kernel_spmd(
        nc, in_maps, core_ids=list(range(N_CORES)), trace=_trace
    )
    outs = [r["out_sb"][invs[c]] for c, r in enumerate(res.results)]
    if _trace:
        kernel.last_results = res
    return np.ascontiguousarray(np.concatenate(outs, axis=0), dtype=np.float32)
